# revision 14
# baseline (speedup 1.0000x reference)
"""Trainium2 Bass kernel for a 2-layer GCN + linear head + log_softmax
(nn_Detector_57604101373957).

Strategy (8 NeuronCores, SPMD):
  - Nodes are dealt to cores by global degree rank (round-robin), then each
    core's 12.5k nodes are degree-sorted into 98 tiles of 128 slots.  Edges
    are partitioned by destination core.  Aggregation uses a round-robin
    layout: round r of tile t gathers the r-th in-edge source row for every
    slot (padded with a dummy zero row), so segment-sum becomes a plain
    [128,64] vector add per round -- no one-hot matmuls.
  - Gathers are 128-row indirect DMAs (one row per partition) from a
    per-core DRAM feature table.
  - Dispatch A: dense transform h=x@W1 (scaled by deg^-1/2) into a per-core
    table, layer-1 aggregation, relu, @W2, scale -> per-core shard of the
    layer-2 gather table.  The host concatenates shards (the "all-gather")
    and launches dispatch B: layer-2 aggregation, relu, head, log-softmax.
"""
import os
import sys
import time

sys.path.insert(0, '/opt/trn_rl_repo')

# This kernel needs the axon-tunneled NeuronCores; undo a cpu-only pin if jax
# hasn't been initialized yet.
_jp = os.environ.get("JAX_PLATFORMS")
if _jp and "axon" not in _jp and "jax" not in sys.modules:
    os.environ.pop("JAX_PLATFORMS", None)

import numpy as np

NCORES = 8
HID = 64
P = 128

_DEBUG = bool(int(os.environ.get("GCN_KERNEL_DEBUG", "0")))


def _log(*a):
    if _DEBUG:
        print("[kernel]", *a, flush=True)


# ----------------------------------------------------------------------------
# toolchain workarounds
# ----------------------------------------------------------------------------
_patched = False


def _apply_patches():
    """This walrus build accepts only ONE semaphore wait per instruction.
    Split Tile's tail-drain waits and any multi-wait instruction onto NOPs."""
    global _patched
    if _patched:
        return
    _patched = True
    import concourse.tile as tile_mod
    from bass_rust import ScopedClock

    def _drain_and_barrier(self, tick_clock, wait_clock):
        nc = self.nc
        import concourse.mybir as mybir
        sink = nc.sync.nop(nofuse=True)
        sink_inst = sink.ins if hasattr(sink, "ins") else sink
        wait_clock.add_sem_waits(sink_inst,
                                ScopedClock({None: tick_clock.global_clock}))
        si = sink_inst.sync_info
        waits = list(si.on_wait) if si is not None and si.on_wait else []
        if len(waits) > 1:
            si.on_wait = waits[:1]
            for k, w in enumerate(waits[1:]):
                extra = nc.sync.nop(nofuse=True)
                extra_inst = extra.ins if hasattr(extra, "ins") else extra
                esi = extra_inst.sync_info
                if esi is None:
                    extra_inst.sync_info = mybir.SyncInfo(on_wait=[w],
                                                          on_update=[])
                else:
                    esi.on_wait = [w]
        nc.sync.drain()
        nc.all_engine_barrier()
        assert self.sems is not None
        popped = nc._tile_sem_poison_stack.pop()
        assert popped is self._sem_poison
        nc.clear_and_free_semaphores(list(self.sems.allocated().values()))
        nc.all_engine_barrier()

    tile_mod.TileContext._drain_and_barrier = _drain_and_barrier


def _split_multi_waits(nc):
    import concourse.mybir as mybir
    n = 0
    for fn in nc.m.functions:
        for bb in fn.blocks:
            new_insts = []
            for inst in bb.instructions:
                si = inst.sync_info
                if si is not None and si.on_wait and len(si.on_wait) > 1:
                    waits = list(si.on_wait)
                    for k, w in enumerate(waits[:-1]):
                        nop = mybir.InstNoOp(
                            name=f"{inst.name}-wsplit{k}",
                            sync_info=mybir.SyncInfo(on_wait=[w],
                                                     on_update=[]),
                            bass_nofuse=True,
                            engine=inst.engine,
                        )
                        new_insts.append(nop)
                    si.on_wait = waits[-1:]
                    n += 1
                new_insts.append(inst)
            bb.instructions[:] = new_insts
    return n


# ----------------------------------------------------------------------------
# SPMD runner (compile once, run; mirrors bass2jax.run_bass_via_pjrt)
# ----------------------------------------------------------------------------
class _Runner:
    def __init__(self, nc, n_cores=NCORES, replicated=()):
        import jax
        from jax.sharding import Mesh, PartitionSpec
        from jax.experimental.shard_map import shard_map
        import concourse.mybir as mybir
        from concourse.bass2jax import (_bass_exec_p, install_neuronx_cc_hook,
                                        partition_id_tensor)
        install_neuronx_cc_hook()
        _split_multi_waits(nc)
        self.jax = jax
        self.n_cores = n_cores
        in_names, out_names, out_avals, zero_outs = [], [], [], []
        pname = nc.partition_id_tensor.name if nc.partition_id_tensor else None
        for alloc in nc.m.functions[0].allocations:
            if not isinstance(alloc, mybir.MemoryLocationSet):
                continue
            name = alloc.memorylocations[0].name
            if alloc.kind == "ExternalInput":
                if name != pname:
                    in_names.append(name)
            elif alloc.kind == "ExternalOutput":
                out_names.append(name)
                shape = tuple(alloc.tensor_shape)
                dtype = mybir.dt.np(alloc.dtype)
                out_avals.append(jax.core.ShapedArray(shape, dtype))
                zero_outs.append(np.zeros(shape, dtype))
        self.in_names, self.out_names = in_names, out_names
        self.out_avals, self.zero_outs = out_avals, zero_outs
        n_params = len(in_names)
        all_in = in_names + out_names + ([pname] if pname else [])

        def _body(*args):
            operands = list(args)
            if pname is not None:
                operands.append(partition_id_tensor())
            return tuple(_bass_exec_p.bind(
                *operands,
                out_avals=tuple(out_avals),
                in_names=tuple(all_in),
                out_names=tuple(out_names),
                lowering_input_output_aliases=(),
                sim_require_finite=True,
                sim_require_nnan=True,
                nc=nc,
            ))

        devices = jax.devices()[:n_cores]
        self.mesh = Mesh(np.asarray(devices), ("core",))
        self.replicated = set(replicated)
        in_specs = tuple(
            PartitionSpec() if name in self.replicated else PartitionSpec("core")
            for name in in_names) + (PartitionSpec("core"),) * len(out_names)
        out_specs = (PartitionSpec("core"),) * len(out_names)
        self.fn = jax.jit(shard_map(_body, mesh=self.mesh, in_specs=in_specs,
                                    out_specs=out_specs, check_rep=False),
                          keep_unused=True)
        self._staged = None

    def stage(self, in_maps):
        from jax.sharding import NamedSharding, PartitionSpec
        n = self.n_cores
        sh = NamedSharding(self.mesh, PartitionSpec("core"))
        shr = NamedSharding(self.mesh, PartitionSpec())
        staged = []
        for name in self.in_names:
            if name in self.replicated:
                staged.append(self.jax.device_put(
                    np.asarray(in_maps[0][name]), shr))
            else:
                staged.append(self.jax.device_put(np.concatenate(
                    [np.asarray(in_maps[c][name]) for c in range(n)], axis=0),
                    sh))
        staged += [self.jax.device_put(
            np.zeros((n * z.shape[0], *z.shape[1:]), z.dtype), sh)
            for z in self.zero_outs]
        self._staged = staged
        self.jax.block_until_ready(self._staged)

    def run(self):
        out = self.fn(*self._staged)
        self.jax.block_until_ready(out)
        n = self.n_cores
        out = [np.asarray(o) for o in out]
        return [{name: out[i].reshape(n, *self.out_avals[i].shape)[c]
                 for i, name in enumerate(self.out_names)}
                for c in range(n)]

    def time_once(self):
        t0 = time.perf_counter()
        out = self.fn(*self._staged)
        self.jax.block_until_ready(out)
        return time.perf_counter() - t0

    def time_pipelined(self, n_iter=10, warmup=2):
        for _ in range(warmup):
            out = self.fn(*self._staged)
        self.jax.block_until_ready(out)
        t0 = time.perf_counter()
        outs = [self.fn(*self._staged) for _ in range(n_iter)]
        self.jax.block_until_ready(outs)
        return (time.perf_counter() - t0) / n_iter


# ----------------------------------------------------------------------------
# host-side graph preparation
# ----------------------------------------------------------------------------
def _prep_graph(edge_index, n):
    src = np.asarray(edge_index[0], dtype=np.int64)
    dst = np.asarray(edge_index[1], dtype=np.int64)
    deg = np.bincount(dst, minlength=n).astype(np.int64) + 1  # + self loop

    # deal nodes to cores by global degree rank
    order = np.argsort(-deg, kind="stable")          # rank -> orig node
    rank_of = np.empty(n, dtype=np.int64)
    rank_of[order] = np.arange(n)
    core_of = rank_of % NCORES
    slot_of = rank_of // NCORES                      # degree-sorted per core

    per_core = (n + NCORES - 1) // NCORES
    tiles = (per_core + P - 1) // P
    slots = tiles * P
    newid = core_of * slots + slot_of                # orig -> new id

    # common per-tile round counts: max degree of slot t*128 across cores
    # (slots are degree-sorted descending, so tile max = first slot's degree)
    D = np.zeros(tiles, dtype=np.int64)
    deg_sorted = deg[order]                          # descending
    for t in range(tiles):
        s0 = t * P
        ranks = s0 * NCORES + np.arange(NCORES)      # first slot of tile t
        ranks = ranks[ranks < n]
        D[t] = deg_sorted[ranks].max() if len(ranks) else 1
    R = int(D.sum())
    offs = np.zeros(tiles, dtype=np.int64)
    offs[1:] = np.cumsum(D)[:-1]

    # in-edge lists grouped by destination (new-id space), self-loops first
    e_order = np.argsort(newid[dst], kind="stable")
    sdst_new = newid[dst][e_order]
    ssrc_new = newid[src][e_order]
    starts = np.searchsorted(sdst_new, np.arange(NCORES * slots), side="left")
    ends = np.searchsorted(sdst_new, np.arange(NCORES * slots), side="right")

    # per-core gather index arrays [128, R] (new-id space), -1 for pad
    nid_grid = np.full((NCORES, slots), -1, dtype=np.int64)
    valid_rank = np.arange(n)
    nid_grid[core_of[order], slot_of[order]] = order  # orig ids on the grid
    idx_new = np.full((NCORES, P, R), -1, dtype=np.int64)
    for t in range(tiles):
        dt = int(D[t])
        o = int(offs[t])
        for c in range(NCORES):
            base = c * slots + t * P
            for p in range(P):
                v = nid_grid[c, t * P + p]
                if v < 0:
                    continue
                nv = base + p
                s, e = starts[nv], ends[nv]
                cnt = e - s
                # self-loop first, then in-edges
                idx_new[c, p, o] = nv
                m = min(cnt, dt - 1)
                idx_new[c, p, o + 1:o + 1 + m] = ssrc_new[s:s + m]
                assert cnt <= dt - 1, (cnt, dt)
    return dict(order=order, newid=newid, core_of=core_of, slot_of=slot_of,
                deg=deg, tiles=tiles, slots=slots, D=D, R=R, offs=offs,
                idx_new=idx_new, nid_grid=nid_grid)


# ----------------------------------------------------------------------------
# bass programs
# ----------------------------------------------------------------------------
def _build_A(tiles_all, tiles, D, R, n_feat):
    """transform + layer-1 aggregation + relu + @W2 + scale -> shard."""
    _skip = set(os.environ.get("GCN_SKIP", "").split(","))
    import concourse.bass as bass
    import concourse.mybir as mybir
    import concourse.tile as tile
    f32, i32 = mybir.dt.float32, mybir.dt.int32
    nall = tiles_all * P
    kt = n_feat // P  # K-tiles of the input feature dim (128 -> 1)
    assert n_feat % P == 0

    nc = bass.Bass()
    xT = nc.dram_tensor("xT", [n_feat, nall], f32, kind="ExternalInput")
    W1d = nc.dram_tensor("W1d", [n_feat, HID], f32, kind="ExternalInput")
    W2d = nc.dram_tensor("W2d", [HID, HID], f32, kind="ExternalInput")
    disall = nc.dram_tensor("disall", [P, tiles_all], f32, kind="ExternalInput")
    disperm = nc.dram_tensor("disperm", [P, tiles], f32, kind="ExternalInput")
    b1rep = nc.dram_tensor("b1rep", [P, HID], f32, kind="ExternalInput")
    idx1 = nc.dram_tensor("idx1", [P, R], i32, kind="ExternalInput")
    identd = nc.dram_tensor("identd", [P, P], f32, kind="ExternalInput")
    shard2 = nc.dram_tensor("shard2", [P, tiles * HID], f32,
                            kind="ExternalOutput")
    table1 = nc.dram_tensor("table1", [nall, HID], f32, kind="Internal")
    t1w = table1[:].rearrange("(p m) d -> p (m d)", p=P)  # p-major write view

    with tile.TileContext(nc) as tc:
        with tc.tile_pool(name="const", bufs=1) as cpool, \
             tc.tile_pool(name="xp", bufs=3) as xpool, \
             tc.tile_pool(name="st", bufs=3) as spool, \
             tc.tile_pool(name="ps", bufs=2, space="PSUM") as pspool, \
             tc.tile_pool(name="msg", bufs=8) as mpool, \
             tc.tile_pool(name="acc", bufs=3) as apool, \
             tc.tile_pool(name="ep", bufs=3) as epool:
            w1 = cpool.tile([n_feat, HID], f32)
            nc.sync.dma_start(out=w1[:], in_=W1d[:, :])
            w2 = cpool.tile([HID, HID], f32)
            nc.sync.dma_start(out=w2[:], in_=W2d[:, :])
            ident = cpool.tile([P, P], f32)
            nc.sync.dma_start(out=ident[:], in_=identd[:, :])
            da = cpool.tile([P, tiles_all], f32)
            nc.sync.dma_start(out=da[:], in_=disall[:, :])
            dp = cpool.tile([P, tiles], f32)
            nc.sync.dma_start(out=dp[:], in_=disperm[:, :])
            b1 = cpool.tile([P, HID], f32)
            nc.sync.dma_start(out=b1[:], in_=b1rep[:, :])
            ix = cpool.tile([P, R], i32)
            nc.sync.dma_start(out=ix[:], in_=idx1[:, :])

            # ---- transform: table1[m*128+p] = dis * (x[.]@W1) ----
            MB = 8 if tiles_all % 8 == 0 else 4
            if "tf" in _skip:
                tiles_all_eff = 0
            else:
                tiles_all_eff = tiles_all
            assert tiles_all % MB == 0, tiles_all
            for mb in range(0, tiles_all_eff, MB):
                xt = xpool.tile([n_feat, MB * P], f32, tag="xt")
                nc.sync.dma_start(out=xt[:], in_=xT[:, mb * P:(mb + MB) * P])
                stage = spool.tile([P, MB, HID], f32, tag="tstage")
                for k in range(MB):
                    ps = pspool.tile([P, HID], f32, tag="tp")
                    nc.tensor.matmul(out=ps[:],
                                     lhsT=xt[:, k * P:(k + 1) * P],
                                     rhs=w1[:], start=True, stop=True)
                    nc.scalar.activation(
                        out=stage[:, k, :], in_=ps[:],
                        func=mybir.ActivationFunctionType.Copy,
                        scale=da[:, mb + k:mb + k + 1])
                nc.sync.dma_start(
                    out=t1w[:, mb * HID:(mb + MB) * HID],
                    in_=stage[:])

            # ---- layer-1 aggregation + batched epilogue + W2 ----
            SB = next(k for k in range(min(14, tiles), 0, -1)
                      if tiles % k == 0)
            GW = next(k for k in (7, 2, 1) if tiles % k == 0 and SB % k == 0)
            offs = [0] * (tiles + 1)
            for t in range(tiles):
                offs[t + 1] = offs[t] + int(D[t])
            for t0 in range(0, tiles, SB):
                wacc = apool.tile([P, SB * HID], f32, tag="wacc")
                for j0 in range(0, SB, GW):
                    base = t0 + j0
                    ds = [int(D[base + j]) for j in range(GW)]
                    assert ds == sorted(ds, reverse=True), ds
                    for j in range(GW):
                        nc.gpsimd.indirect_dma_start(
                            out=wacc[:, (j0 + j) * HID:(j0 + j + 1) * HID],
                            out_offset=None, in_=table1[:],
                            in_offset=bass.IndirectOffsetOnAxis(
                                ap=ix[:, offs[base + j]:offs[base + j] + 1],
                                axis=0))
                    for r in range(1, ds[0]):
                        w = sum(1 for d in ds if d > r)
                        msgw = mpool.tile([P, GW * HID], f32, tag="msgw")
                        for j in range(w):
                            nc.gpsimd.indirect_dma_start(
                                out=msgw[:, j * HID:(j + 1) * HID],
                                out_offset=None, in_=table1[:],
                                in_offset=bass.IndirectOffsetOnAxis(
                                    ap=ix[:, offs[base + j] + r:
                                           offs[base + j] + r + 1], axis=0))
                        gsl = wacc[:, j0 * HID:(j0 + w) * HID]
                        nc.vector.tensor_tensor(out=gsl, in0=gsl,
                                                in1=msgw[:, :w * HID],
                                                op=mybir.AluOpType.add)
                # batched epilogue: h1 = relu(wacc*dis + b1) over SB tiles
                h1w = epool.tile([P, SB * HID], f32, tag="h1w")
                h1w3 = h1w[:].rearrange("p (s d) -> p s d", d=HID)
                nc.vector.tensor_tensor(
                    out=h1w3, in0=wacc[:].rearrange("p (s d) -> p s d", d=HID),
                    in1=dp[:, t0:t0 + SB].to_broadcast([P, SB, HID]),
                    op=mybir.AluOpType.mult)
                nc.vector.tensor_tensor(
                    out=h1w3, in0=h1w3,
                    in1=b1[:].rearrange("p (o d) -> p o d", o=1)
                          .to_broadcast([P, SB, HID]),
                    op=mybir.AluOpType.add)
                nc.scalar.activation(out=h1w[:], in_=h1w[:],
                                     func=mybir.ActivationFunctionType.Relu)
                stage2 = spool.tile([P, SB, HID], f32, tag="sstage")
                for j0 in range(0, SB, GW):
                    p2w = pspool.tile([P, GW * HID], f32, tag="p2w")
                    for j in range(GW):
                        t = t0 + j0 + j
                        pT = pspool.tile([HID, P], f32, tag="pT")
                        nc.tensor.transpose(
                            out=pT[:],
                            in_=h1w[:, (j0 + j) * HID:(j0 + j + 1) * HID],
                            identity=ident[:])
                        lT = epool.tile([HID, P], f32, tag="lT")
                        nc.vector.tensor_copy(out=lT[:], in_=pT[:])
                        nc.tensor.matmul(out=p2w[:, j * HID:(j + 1) * HID],
                                         lhsT=lT[:], rhs=w2[:],
                                         start=True, stop=True)
                    nc.vector.tensor_tensor(
                        out=stage2[:, j0:j0 + GW, :],
                        in0=p2w[:].rearrange("p (s d) -> p s d", d=HID),
                        in1=dp[:, t0 + j0:t0 + j0 + GW]
                              .to_broadcast([P, GW, HID]),
                        op=mybir.AluOpType.mult)
                nc.sync.dma_start(
                    out=shard2[:, t0 * HID:(t0 + SB) * HID],
                    in_=stage2[:])
    return nc


def _build_B(tiles, D, R, vt2, b3diff):
    """layer-2 aggregation + relu + head + log_softmax.

    The core's own shard rows (= every node's self-loop contribution) are
    preloaded as a wide SBUF accumulator, so round 0 needs no gathers.
    Output layout: outd[:, 0:tiles] = lp0, outd[:, tiles:2*tiles] = lp1
    (p-major per tile)."""
    import concourse.bass as bass
    import concourse.mybir as mybir
    import concourse.tile as tile
    f32, i32 = mybir.dt.float32, mybir.dt.int32

    nc = bass.Bass()
    table2 = nc.dram_tensor("table2", [vt2, HID], f32, kind="ExternalInput")
    idx2 = nc.dram_tensor("idx2", [P, R], i32, kind="ExternalInput")
    disperm = nc.dram_tensor("disperm", [P, tiles], f32, kind="ExternalInput")
    b2rep = nc.dram_tensor("b2rep", [P, HID], f32, kind="ExternalInput")
    w3drep = nc.dram_tensor("w3drep", [P, HID], f32, kind="ExternalInput")
    own2d = nc.dram_tensor("own2d", [P, tiles * HID], f32,
                           kind="ExternalInput")
    outd = nc.dram_tensor("out", [P, 2 * tiles], f32, kind="ExternalOutput")

    with tile.TileContext(nc) as tc:
        with tc.tile_pool(name="const", bufs=1) as cpool, \
             tc.tile_pool(name="msg", bufs=8) as mpool, \
             tc.tile_pool(name="ep", bufs=4) as epool:
            dp = cpool.tile([P, tiles], f32)
            nc.sync.dma_start(out=dp[:], in_=disperm[:, :])
            b2 = cpool.tile([P, HID], f32)
            nc.sync.dma_start(out=b2[:], in_=b2rep[:, :])
            w3 = cpool.tile([P, HID], f32)
            nc.sync.dma_start(out=w3[:], in_=w3drep[:, :])
            ix = cpool.tile([P, R], i32)
            nc.sync.dma_start(out=ix[:], in_=idx2[:, :])
            own = cpool.tile([P, tiles * HID], f32)
            nc.sync.dma_start(out=own[:], in_=own2d[:, :])

            SB = next(k for k in range(min(14, tiles), 0, -1)
                      if tiles % k == 0)
            GW = next(k for k in (7, 2, 1) if tiles % k == 0 and SB % k == 0)
            offs = [0] * (tiles + 1)
            for t in range(tiles):
                offs[t + 1] = offs[t] + int(D[t])
            for t0 in range(0, tiles, SB):
                for j0 in range(0, SB, GW):
                    base = t0 + j0
                    ds = [int(D[base + j]) for j in range(GW)]
                    assert ds == sorted(ds, reverse=True), ds
                    for r in range(1, ds[0]):
                        w = sum(1 for d in ds if d > r)
                        msgw = mpool.tile([P, GW * HID], f32, tag="msgw")
                        for j in range(w):
                            nc.gpsimd.indirect_dma_start(
                                out=msgw[:, j * HID:(j + 1) * HID],
                                out_offset=None, in_=table2[:],
                                in_offset=bass.IndirectOffsetOnAxis(
                                    ap=ix[:, offs[base + j] + r:
                                           offs[base + j] + r + 1], axis=0))
                        gsl = own[:, base * HID:(base + w) * HID]
                        nc.vector.tensor_tensor(out=gsl, in0=gsl,
                                                in1=msgw[:, :w * HID],
                                                op=mybir.AluOpType.add)
                # batched epilogue over SB tiles
                seg3 = own[:, t0 * HID:(t0 + SB) * HID].rearrange(
                    "p (s d) -> p s d", d=HID)
                o2 = epool.tile([P, SB * HID], f32, tag="o2")
                o23 = o2[:].rearrange("p (s d) -> p s d", d=HID)
                nc.vector.tensor_tensor(
                    out=o23, in0=seg3,
                    in1=dp[:, t0:t0 + SB].to_broadcast([P, SB, HID]),
                    op=mybir.AluOpType.mult)
                nc.vector.tensor_tensor(
                    out=o23, in0=o23,
                    in1=b2[:].rearrange("p (o d) -> p o d", o=1)
                          .to_broadcast([P, SB, HID]),
                    op=mybir.AluOpType.add)
                nc.scalar.activation(out=o2[:], in_=o2[:],
                                     func=mybir.ActivationFunctionType.Relu)
                # head: z = sum_d o2*w3diff (+ b3diff); lp0=-sp(z); lp1=z-sp(z)
                tmp = epool.tile([P, SB * HID], f32, tag="tmp")
                nc.vector.tensor_tensor(
                    out=tmp[:].rearrange("p (s d) -> p s d", d=HID), in0=o23,
                    in1=w3[:].rearrange("p (o d) -> p o d", o=1)
                          .to_broadcast([P, SB, HID]),
                    op=mybir.AluOpType.mult)
                z = epool.tile([P, SB], f32, tag="z")
                nc.vector.tensor_reduce(
                    out=z[:].rearrange("p (s o) -> p s o", o=1),
                    in_=tmp[:].rearrange("p (s d) -> p s d", d=HID),
                    axis=mybir.AxisListType.X, op=mybir.AluOpType.add)
                if b3diff != 0.0:
                    nc.vector.tensor_scalar_add(out=z[:], in0=z[:],
                                                scalar1=float(b3diff))
                # stable softplus: m=max(z,0); sp = m + ln(1+exp(z-2m))
                m = epool.tile([P, SB], f32, tag="m")
                nc.vector.tensor_scalar_max(out=m[:], in0=z[:], scalar1=0.0)
                e = epool.tile([P, SB], f32, tag="e")
                nc.vector.tensor_scalar(out=e[:], in0=m[:], scalar1=-2.0,
                                        scalar2=None,
                                        op0=mybir.AluOpType.mult)
                nc.vector.tensor_tensor(out=e[:], in0=e[:], in1=z[:],
                                        op=mybir.AluOpType.add)
                nc.scalar.activation(out=e[:], in_=e[:],
                                     func=mybir.ActivationFunctionType.Exp)
                nc.vector.tensor_scalar_add(out=e[:], in0=e[:], scalar1=1.0)
                nc.scalar.activation(out=e[:], in_=e[:],
                                     func=mybir.ActivationFunctionType.Ln)
                sp = epool.tile([P, SB], f32, tag="sp")
                nc.vector.tensor_tensor(out=sp[:], in0=e[:], in1=m[:],
                                        op=mybir.AluOpType.add)
                lp0 = epool.tile([P, SB], f32, tag="lp0")
                nc.vector.tensor_scalar_mul(out=lp0[:], in0=sp[:],
                                            scalar1=-1.0)
                lp1 = epool.tile([P, SB], f32, tag="lp1")
                nc.vector.tensor_tensor(out=lp1[:], in0=z[:], in1=sp[:],
                                        op=mybir.AluOpType.subtract)
                nc.sync.dma_start(out=outd[:, t0:t0 + SB], in_=lp0[:])
                nc.sync.dma_start(out=outd[:, tiles + t0:tiles + t0 + SB],
                                  in_=lp1[:])
    return nc


# ----------------------------------------------------------------------------
# main entry
# ----------------------------------------------------------------------------
def kernel(x, edge_index, W1, b1, W2, b2, W3, b3):
    _apply_patches()
    x = np.asarray(x, dtype=np.float32)
    n, n_feat = x.shape
    t_start = time.time()
    g = _prep_graph(edge_index, n)
    tiles, slots, D, R = g["tiles"], g["slots"], g["D"], g["R"]
    tiles_all = NCORES * tiles
    nall = tiles_all * P
    _log(f"prep {time.time()-t_start:.1f}s tiles={tiles} R={R} "
         f"pad={(R*P*NCORES)/(edge_index.shape[1]+n)-1:.2%}")

    order, newid = g["order"], g["newid"]
    deg = g["deg"]
    dis = (1.0 / np.sqrt(deg)).astype(np.float32)

    # xT_perm [n_feat, nall]: column (new id) = x[orig]; pads zero
    xTp = np.zeros((n_feat, nall), dtype=np.float32)
    xTp[:, newid] = x.T
    # dis arrays
    disall = np.ones((P, tiles_all), dtype=np.float32)
    disall[newid % P, newid // P] = dis
    disperm = [np.ones((P, tiles), dtype=np.float32) for _ in range(NCORES)]
    for c in range(NCORES):
        nid = g["nid_grid"][c]  # orig ids per slot, -1 pad
        m = nid >= 0
        s = np.arange(slots)[m]
        disperm[c][s % P, s // P] = dis[nid[m]]
    b1rep = np.broadcast_to(np.asarray(b1, np.float32), (P, HID)).copy()
    b2rep = np.broadcast_to(np.asarray(b2, np.float32), (P, HID)).copy()
    w3 = np.asarray(W3, np.float32)
    w3drep = np.broadcast_to((w3[:, 1] - w3[:, 0]), (P, HID)).copy()
    b3 = np.asarray(b3, np.float32)
    b3diff = float(b3[1] - b3[0])

    # gather indices
    idx_new = g["idx_new"]  # [NCORES, P, R] new-ids, -1 pad
    DUMMY1_NEW = slots - 1  # core 0's last pad slot -> zero row (pads exist
    # because slots > n/NCORES; guaranteed since 12544 > 12500)
    assert slots * NCORES > n, "no pad slot available for dummy row"
    i1 = idx_new.copy()
    i1[i1 < 0] = DUMMY1_NEW
    # p-major table1 mapping: row(nid) = (nid%128)*tiles_all + nid//128
    idx1 = ((i1 % P) * tiles_all + i1 // P).astype(np.int32)
    vt2 = NCORES * slots + 1
    i2 = idx_new.copy()
    i2[i2 < 0] = vt2 - 1  # host-appended zero row
    idx2 = i2.astype(np.int32)

    _log(f"host arrays {time.time()-t_start:.1f}s")

    # ---- dispatch A ----
    ncA = _build_A(tiles_all, tiles, D, R, n_feat)
    _log(f"built A {time.time()-t_start:.1f}s")
    rA = _Runner(ncA, replicated=("xT", "W1d", "W2d", "disall", "b1rep",
                                  "identd"))
    inA = [{"xT": xTp,
            "W1d": np.asarray(W1, np.float32),
            "W2d": np.asarray(W2, np.float32),
            "disall": disall,
            "disperm": disperm[c],
            "b1rep": b1rep,
            "identd": np.eye(P, dtype=np.float32),
            "idx1": idx1[c]} for c in range(NCORES)]
    rA.stage(inA)
    _log(f"staged A {time.time()-t_start:.1f}s")
    resA = rA.run()
    _log(f"ran A {time.time()-t_start:.1f}s")

    # host all-gather: assemble table2 [vt2, HID]
    table2 = np.zeros((vt2, HID), dtype=np.float32)
    for c in range(NCORES):
        sh = resA[c]["shard2"].reshape(P, tiles, HID).transpose(1, 0, 2)
        table2[c * slots:(c + 1) * slots] = sh.reshape(slots, HID)
    # zero the pad rows (robustness when b1 != 0)
    pad_mask = np.ones(NCORES * slots, dtype=bool)
    for c in range(NCORES):
        nid = g["nid_grid"][c]
        s = np.arange(slots)[nid >= 0]
        pad_mask[c * slots + s] = False
    table2[:-1][pad_mask] = 0.0

    # ---- dispatch B ----
    ncB = _build_B(tiles, D, R, vt2, b3diff)
    _log(f"built B {time.time()-t_start:.1f}s")
    rB = _Runner(ncB, replicated=("table2", "b2rep", "w3drep"))
    inB = [{"table2": table2,
            "idx2": idx2[c],
            "disperm": disperm[c],
            "b2rep": b2rep,
            "w3drep": w3drep,
            "own2d": resA[c]["shard2"]} for c in range(NCORES)]
    rB.stage(inB)
    _log(f"staged B {time.time()-t_start:.1f}s")
    resB = rB.run()
    _log(f"ran B {time.time()-t_start:.1f}s")

    # ---- unshard: [P, tiles*2] p-major -> [n, 2] in original order ----
    full = np.empty((NCORES * slots, 2), dtype=np.float32)
    for c in range(NCORES):
        o = resB[c]["out"]  # [P, 2*tiles]: lp0 block | lp1 block, p-major
        lp0 = o[:, :tiles].T.reshape(slots)
        lp1 = o[:, tiles:].T.reshape(slots)
        full[c * slots:(c + 1) * slots, 0] = lp0
        full[c * slots:(c + 1) * slots, 1] = lp1
    out = full[newid]
    # keep runners alive for optional re-timing by test harness
    kernel._last = dict(rA=rA, rB=rB)
    _log(f"done {time.time()-t_start:.1f}s")
    return out.astype(np.float32)


# revision 15
# speedup vs baseline: 1.4158x; 1.4158x over previous
"""Trainium2 Bass kernel for a 2-layer GCN + linear head + log_softmax
(nn_Detector_57604101373957).

Strategy (8 NeuronCores, SPMD):
  - Nodes are dealt to cores by global degree rank (round-robin), then each
    core's 12.5k nodes are degree-sorted into 98 tiles of 128 slots.  Edges
    are partitioned by destination core.  Aggregation uses a round-robin
    layout: round r of tile t gathers the r-th in-edge source row for every
    slot (padded with a dummy zero row), so segment-sum becomes a plain
    [128,64] vector add per round -- no one-hot matmuls.
  - Gathers are 128-row indirect DMAs (one row per partition) from a
    per-core DRAM feature table.
  - Dispatch A: dense transform h=x@W1 (scaled by deg^-1/2) into a per-core
    table, layer-1 aggregation, relu, @W2, scale -> per-core shard of the
    layer-2 gather table.  The host concatenates shards (the "all-gather")
    and launches dispatch B: layer-2 aggregation, relu, head, log-softmax.
"""
import os
import sys
import time

sys.path.insert(0, '/opt/trn_rl_repo')

# This kernel needs the axon-tunneled NeuronCores; undo a cpu-only pin if jax
# hasn't been initialized yet.
_jp = os.environ.get("JAX_PLATFORMS")
if _jp and "axon" not in _jp and "jax" not in sys.modules:
    os.environ.pop("JAX_PLATFORMS", None)

import numpy as np

NCORES = 8
HID = 64
P = 128

_DEBUG = bool(int(os.environ.get("GCN_KERNEL_DEBUG", "0")))


def _log(*a):
    if _DEBUG:
        print("[kernel]", *a, flush=True)


# ----------------------------------------------------------------------------
# toolchain workarounds
# ----------------------------------------------------------------------------
_patched = False


def _apply_patches():
    """This walrus build accepts only ONE semaphore wait per instruction.
    Split Tile's tail-drain waits and any multi-wait instruction onto NOPs."""
    global _patched
    if _patched:
        return
    _patched = True
    import concourse.tile as tile_mod
    from bass_rust import ScopedClock

    def _drain_and_barrier(self, tick_clock, wait_clock):
        nc = self.nc
        import concourse.mybir as mybir
        sink = nc.sync.nop(nofuse=True)
        sink_inst = sink.ins if hasattr(sink, "ins") else sink
        wait_clock.add_sem_waits(sink_inst,
                                ScopedClock({None: tick_clock.global_clock}))
        si = sink_inst.sync_info
        waits = list(si.on_wait) if si is not None and si.on_wait else []
        if len(waits) > 1:
            si.on_wait = waits[:1]
            for k, w in enumerate(waits[1:]):
                extra = nc.sync.nop(nofuse=True)
                extra_inst = extra.ins if hasattr(extra, "ins") else extra
                esi = extra_inst.sync_info
                if esi is None:
                    extra_inst.sync_info = mybir.SyncInfo(on_wait=[w],
                                                          on_update=[])
                else:
                    esi.on_wait = [w]
        nc.sync.drain()
        nc.all_engine_barrier()
        assert self.sems is not None
        popped = nc._tile_sem_poison_stack.pop()
        assert popped is self._sem_poison
        nc.clear_and_free_semaphores(list(self.sems.allocated().values()))
        nc.all_engine_barrier()

    tile_mod.TileContext._drain_and_barrier = _drain_and_barrier


def _split_multi_waits(nc):
    import concourse.mybir as mybir
    n = 0
    for fn in nc.m.functions:
        for bb in fn.blocks:
            new_insts = []
            for inst in bb.instructions:
                si = inst.sync_info
                if si is not None and si.on_wait and len(si.on_wait) > 1:
                    waits = list(si.on_wait)
                    for k, w in enumerate(waits[:-1]):
                        nop = mybir.InstNoOp(
                            name=f"{inst.name}-wsplit{k}",
                            sync_info=mybir.SyncInfo(on_wait=[w],
                                                     on_update=[]),
                            bass_nofuse=True,
                            engine=inst.engine,
                        )
                        new_insts.append(nop)
                    si.on_wait = waits[-1:]
                    n += 1
                new_insts.append(inst)
            bb.instructions[:] = new_insts
    return n


# ----------------------------------------------------------------------------
# SPMD runner (compile once, run; mirrors bass2jax.run_bass_via_pjrt)
# ----------------------------------------------------------------------------
class _Runner:
    def __init__(self, nc, n_cores=NCORES, replicated=()):
        import jax
        from jax.sharding import Mesh, PartitionSpec
        from jax.experimental.shard_map import shard_map
        import concourse.mybir as mybir
        from concourse.bass2jax import (_bass_exec_p, install_neuronx_cc_hook,
                                        partition_id_tensor)
        install_neuronx_cc_hook()
        _split_multi_waits(nc)
        self.jax = jax
        self.n_cores = n_cores
        in_names, out_names, out_avals, zero_outs = [], [], [], []
        pname = nc.partition_id_tensor.name if nc.partition_id_tensor else None
        for alloc in nc.m.functions[0].allocations:
            if not isinstance(alloc, mybir.MemoryLocationSet):
                continue
            name = alloc.memorylocations[0].name
            if alloc.kind == "ExternalInput":
                if name != pname:
                    in_names.append(name)
            elif alloc.kind == "ExternalOutput":
                out_names.append(name)
                shape = tuple(alloc.tensor_shape)
                dtype = mybir.dt.np(alloc.dtype)
                out_avals.append(jax.core.ShapedArray(shape, dtype))
                zero_outs.append(np.zeros(shape, dtype))
        self.in_names, self.out_names = in_names, out_names
        self.out_avals, self.zero_outs = out_avals, zero_outs
        n_params = len(in_names)
        all_in = in_names + out_names + ([pname] if pname else [])

        def _body(*args):
            operands = list(args)
            if pname is not None:
                operands.append(partition_id_tensor())
            return tuple(_bass_exec_p.bind(
                *operands,
                out_avals=tuple(out_avals),
                in_names=tuple(all_in),
                out_names=tuple(out_names),
                lowering_input_output_aliases=(),
                sim_require_finite=True,
                sim_require_nnan=True,
                nc=nc,
            ))

        devices = jax.devices()[:n_cores]
        self.mesh = Mesh(np.asarray(devices), ("core",))
        self.replicated = set(replicated)
        in_specs = tuple(
            PartitionSpec() if name in self.replicated else PartitionSpec("core")
            for name in in_names) + (PartitionSpec("core"),) * len(out_names)
        out_specs = (PartitionSpec("core"),) * len(out_names)
        self.fn = jax.jit(shard_map(_body, mesh=self.mesh, in_specs=in_specs,
                                    out_specs=out_specs, check_rep=False),
                          keep_unused=True)
        self._staged = None

    def stage(self, in_maps):
        from jax.sharding import NamedSharding, PartitionSpec
        n = self.n_cores
        sh = NamedSharding(self.mesh, PartitionSpec("core"))
        shr = NamedSharding(self.mesh, PartitionSpec())
        staged = []
        for name in self.in_names:
            if name in self.replicated:
                staged.append(self.jax.device_put(
                    np.asarray(in_maps[0][name]), shr))
            else:
                staged.append(self.jax.device_put(np.concatenate(
                    [np.asarray(in_maps[c][name]) for c in range(n)], axis=0),
                    sh))
        staged += [self.jax.device_put(
            np.zeros((n * z.shape[0], *z.shape[1:]), z.dtype), sh)
            for z in self.zero_outs]
        self._staged = staged
        self.jax.block_until_ready(self._staged)

    def run(self):
        out = self.fn(*self._staged)
        self.jax.block_until_ready(out)
        n = self.n_cores
        out = [np.asarray(o) for o in out]
        return [{name: out[i].reshape(n, *self.out_avals[i].shape)[c]
                 for i, name in enumerate(self.out_names)}
                for c in range(n)]

    def time_once(self):
        t0 = time.perf_counter()
        out = self.fn(*self._staged)
        self.jax.block_until_ready(out)
        return time.perf_counter() - t0

    def time_pipelined(self, n_iter=10, warmup=2):
        for _ in range(warmup):
            out = self.fn(*self._staged)
        self.jax.block_until_ready(out)
        t0 = time.perf_counter()
        outs = [self.fn(*self._staged) for _ in range(n_iter)]
        self.jax.block_until_ready(outs)
        return (time.perf_counter() - t0) / n_iter


# ----------------------------------------------------------------------------
# host-side graph preparation
# ----------------------------------------------------------------------------
def _prep_graph(edge_index, n):
    src = np.asarray(edge_index[0], dtype=np.int64)
    dst = np.asarray(edge_index[1], dtype=np.int64)
    deg = np.bincount(dst, minlength=n).astype(np.int64) + 1  # + self loop

    # deal nodes to cores by global degree rank
    order = np.argsort(-deg, kind="stable")          # rank -> orig node
    rank_of = np.empty(n, dtype=np.int64)
    rank_of[order] = np.arange(n)
    core_of = rank_of % NCORES
    slot_of = rank_of // NCORES                      # degree-sorted per core

    per_core = (n + NCORES - 1) // NCORES
    tiles = (per_core + P - 1) // P
    slots = tiles * P
    newid = core_of * slots + slot_of                # orig -> new id

    # common per-tile round counts: max degree of slot t*128 across cores
    # (slots are degree-sorted descending, so tile max = first slot's degree)
    D = np.zeros(tiles, dtype=np.int64)
    deg_sorted = deg[order]                          # descending
    for t in range(tiles):
        s0 = t * P
        ranks = s0 * NCORES + np.arange(NCORES)      # first slot of tile t
        ranks = ranks[ranks < n]
        D[t] = deg_sorted[ranks].max() if len(ranks) else 1
    R = int(D.sum())
    offs = np.zeros(tiles, dtype=np.int64)
    offs[1:] = np.cumsum(D)[:-1]

    # in-edge lists grouped by destination (new-id space), self-loops first
    e_order = np.argsort(newid[dst], kind="stable")
    sdst_new = newid[dst][e_order]
    ssrc_new = newid[src][e_order]
    starts = np.searchsorted(sdst_new, np.arange(NCORES * slots), side="left")
    ends = np.searchsorted(sdst_new, np.arange(NCORES * slots), side="right")

    # per-core gather index arrays [128, R] (new-id space), -1 for pad
    nid_grid = np.full((NCORES, slots), -1, dtype=np.int64)
    valid_rank = np.arange(n)
    nid_grid[core_of[order], slot_of[order]] = order  # orig ids on the grid
    idx_new = np.full((NCORES, P, R), -1, dtype=np.int64)
    for t in range(tiles):
        dt = int(D[t])
        o = int(offs[t])
        for c in range(NCORES):
            base = c * slots + t * P
            for p in range(P):
                v = nid_grid[c, t * P + p]
                if v < 0:
                    continue
                nv = base + p
                s, e = starts[nv], ends[nv]
                cnt = e - s
                # self-loop first, then in-edges
                idx_new[c, p, o] = nv
                m = min(cnt, dt - 1)
                idx_new[c, p, o + 1:o + 1 + m] = ssrc_new[s:s + m]
                assert cnt <= dt - 1, (cnt, dt)
    return dict(order=order, newid=newid, core_of=core_of, slot_of=slot_of,
                deg=deg, tiles=tiles, slots=slots, D=D, R=R, offs=offs,
                idx_new=idx_new, nid_grid=nid_grid)


# ----------------------------------------------------------------------------
# bass programs
# ----------------------------------------------------------------------------
def _build_A(tiles_all, tiles, D, R, n_feat):
    """transform + layer-1 aggregation + relu + @W2 + scale -> shard."""
    _skip = set(os.environ.get("GCN_SKIP", "").split(","))
    import concourse.bass as bass
    import concourse.mybir as mybir
    import concourse.tile as tile
    f32, i32 = mybir.dt.float32, mybir.dt.int32
    nall = tiles_all * P
    kt = n_feat // P  # K-tiles of the input feature dim (128 -> 1)
    assert n_feat % P == 0

    nc = bass.Bass()
    xT = nc.dram_tensor("xT", [n_feat, nall], f32, kind="ExternalInput")
    W1d = nc.dram_tensor("W1d", [n_feat, HID], f32, kind="ExternalInput")
    W2d = nc.dram_tensor("W2d", [HID, HID], f32, kind="ExternalInput")
    disall = nc.dram_tensor("disall", [P, tiles_all], f32, kind="ExternalInput")
    disperm = nc.dram_tensor("disperm", [P, tiles], f32, kind="ExternalInput")
    b1rep = nc.dram_tensor("b1rep", [P, HID], f32, kind="ExternalInput")
    idx1 = nc.dram_tensor("idx1", [P, R], i32, kind="ExternalInput")
    identd = nc.dram_tensor("identd", [P, P], f32, kind="ExternalInput")
    shard2 = nc.dram_tensor("shard2", [P, tiles * HID], f32,
                            kind="ExternalOutput")
    table1 = nc.dram_tensor("table1", [nall, HID], f32, kind="Internal")
    t1w = table1[:].rearrange("(p m) d -> p (m d)", p=P)  # p-major write view

    with tile.TileContext(nc) as tc:
        with tc.tile_pool(name="const", bufs=1) as cpool, \
             tc.tile_pool(name="xp", bufs=3) as xpool, \
             tc.tile_pool(name="st", bufs=3) as spool, \
             tc.tile_pool(name="ps", bufs=2, space="PSUM") as pspool, \
             tc.tile_pool(name="msg", bufs=8) as mpool, \
             tc.tile_pool(name="acc", bufs=3) as apool, \
             tc.tile_pool(name="ep", bufs=3) as epool:
            w1 = cpool.tile([n_feat, HID], f32)
            nc.sync.dma_start(out=w1[:], in_=W1d[:, :])
            w2 = cpool.tile([HID, HID], f32)
            nc.sync.dma_start(out=w2[:], in_=W2d[:, :])
            ident = cpool.tile([P, P], f32)
            nc.sync.dma_start(out=ident[:], in_=identd[:, :])
            da = cpool.tile([P, tiles_all], f32)
            nc.sync.dma_start(out=da[:], in_=disall[:, :])
            dp = cpool.tile([P, tiles], f32)
            nc.sync.dma_start(out=dp[:], in_=disperm[:, :])
            b1 = cpool.tile([P, HID], f32)
            nc.sync.dma_start(out=b1[:], in_=b1rep[:, :])
            ix = cpool.tile([P, R], i32)
            nc.sync.dma_start(out=ix[:], in_=idx1[:, :])

            # ---- transform: table1[m*128+p] = dis * (x[.]@W1) ----
            MB = 8 if tiles_all % 8 == 0 else 4
            if "tf" in _skip:
                tiles_all_eff = 0
            else:
                tiles_all_eff = tiles_all
            assert tiles_all % MB == 0, tiles_all
            for mb in range(0, tiles_all_eff, MB):
                xt = xpool.tile([n_feat, MB * P], f32, tag="xt")
                nc.sync.dma_start(out=xt[:], in_=xT[:, mb * P:(mb + MB) * P])
                stage = spool.tile([P, MB, HID], f32, tag="tstage")
                for k in range(MB):
                    ps = pspool.tile([P, HID], f32, tag="tp")
                    nc.tensor.matmul(out=ps[:],
                                     lhsT=xt[:, k * P:(k + 1) * P],
                                     rhs=w1[:], start=True, stop=True)
                    nc.scalar.activation(
                        out=stage[:, k, :], in_=ps[:],
                        func=mybir.ActivationFunctionType.Copy,
                        scale=da[:, mb + k:mb + k + 1])
                nc.sync.dma_start(
                    out=t1w[:, mb * HID:(mb + MB) * HID],
                    in_=stage[:])

            # ---- layer-1 aggregation + batched epilogue + W2 ----
            SB = next(k for k in range(min(14, tiles), 0, -1)
                      if tiles % k == 0)
            GW = next(k for k in (7, 2, 1) if tiles % k == 0 and SB % k == 0)
            offs = [0] * (tiles + 1)
            for t in range(tiles):
                offs[t + 1] = offs[t] + int(D[t])
            # round 0 of every tile is the self-loop: this core's own rows,
            # which sit at a contiguous column slice of the p-major table
            # view at a partition-id-dependent offset.  One dynamic-slice DMA
            # preloads them all as the accumulator (cf. dispatch B's own2d).
            pid = nc.sync.partition_id()
            own1 = cpool.tile([P, tiles * HID], f32)
            nc.sync.dma_start(
                out=own1[:],
                in_=t1w[:, bass.ds(pid * (tiles * HID), tiles * HID)])
            for t0 in range(0, tiles, SB):
                for j0 in range(0, SB, GW):
                    base = t0 + j0
                    ds = [int(D[base + j]) for j in range(GW)]
                    assert ds == sorted(ds, reverse=True), ds
                    for r in range(1, ds[0]):
                        w = sum(1 for d in ds if d > r)
                        msgw = mpool.tile([P, GW * HID], f32, tag="msgw")
                        for j in range(w):
                            nc.gpsimd.indirect_dma_start(
                                out=msgw[:, j * HID:(j + 1) * HID],
                                out_offset=None, in_=table1[:],
                                in_offset=bass.IndirectOffsetOnAxis(
                                    ap=ix[:, offs[base + j] + r:
                                           offs[base + j] + r + 1], axis=0))
                        gsl = own1[:, base * HID:(base + w) * HID]
                        nc.vector.tensor_tensor(out=gsl, in0=gsl,
                                                in1=msgw[:, :w * HID],
                                                op=mybir.AluOpType.add)
                # batched epilogue: h1 = relu(own1*dis + b1) over SB tiles
                h1w = epool.tile([P, SB * HID], f32, tag="h1w")
                h1w3 = h1w[:].rearrange("p (s d) -> p s d", d=HID)
                nc.vector.tensor_tensor(
                    out=h1w3,
                    in0=own1[:, t0 * HID:(t0 + SB) * HID]
                          .rearrange("p (s d) -> p s d", d=HID),
                    in1=dp[:, t0:t0 + SB].to_broadcast([P, SB, HID]),
                    op=mybir.AluOpType.mult)
                nc.vector.tensor_tensor(
                    out=h1w3, in0=h1w3,
                    in1=b1[:].rearrange("p (o d) -> p o d", o=1)
                          .to_broadcast([P, SB, HID]),
                    op=mybir.AluOpType.add)
                nc.scalar.activation(out=h1w[:], in_=h1w[:],
                                     func=mybir.ActivationFunctionType.Relu)
                stage2 = spool.tile([P, SB, HID], f32, tag="sstage")
                for j0 in range(0, SB, GW):
                    p2w = pspool.tile([P, GW * HID], f32, tag="p2w")
                    for j in range(GW):
                        t = t0 + j0 + j
                        pT = pspool.tile([HID, P], f32, tag="pT")
                        nc.tensor.transpose(
                            out=pT[:],
                            in_=h1w[:, (j0 + j) * HID:(j0 + j + 1) * HID],
                            identity=ident[:])
                        lT = epool.tile([HID, P], f32, tag="lT")
                        nc.vector.tensor_copy(out=lT[:], in_=pT[:])
                        nc.tensor.matmul(out=p2w[:, j * HID:(j + 1) * HID],
                                         lhsT=lT[:], rhs=w2[:],
                                         start=True, stop=True)
                    nc.vector.tensor_tensor(
                        out=stage2[:, j0:j0 + GW, :],
                        in0=p2w[:].rearrange("p (s d) -> p s d", d=HID),
                        in1=dp[:, t0 + j0:t0 + j0 + GW]
                              .to_broadcast([P, GW, HID]),
                        op=mybir.AluOpType.mult)
                nc.sync.dma_start(
                    out=shard2[:, t0 * HID:(t0 + SB) * HID],
                    in_=stage2[:])
    return nc


def _build_B(tiles, D, R, vt2, b3diff):
    """layer-2 aggregation + relu + head + log_softmax.

    The core's own shard rows (= every node's self-loop contribution) are
    preloaded as a wide SBUF accumulator, so round 0 needs no gathers.
    Output layout: outd[:, 0:tiles] = lp0, outd[:, tiles:2*tiles] = lp1
    (p-major per tile)."""
    import concourse.bass as bass
    import concourse.mybir as mybir
    import concourse.tile as tile
    f32, i32 = mybir.dt.float32, mybir.dt.int32

    nc = bass.Bass()
    table2 = nc.dram_tensor("table2", [vt2, HID], f32, kind="ExternalInput")
    idx2 = nc.dram_tensor("idx2", [P, R], i32, kind="ExternalInput")
    disperm = nc.dram_tensor("disperm", [P, tiles], f32, kind="ExternalInput")
    b2rep = nc.dram_tensor("b2rep", [P, HID], f32, kind="ExternalInput")
    w3drep = nc.dram_tensor("w3drep", [P, HID], f32, kind="ExternalInput")
    own2d = nc.dram_tensor("own2d", [P, tiles * HID], f32,
                           kind="ExternalInput")
    outd = nc.dram_tensor("out", [P, 2 * tiles], f32, kind="ExternalOutput")

    with tile.TileContext(nc) as tc:
        with tc.tile_pool(name="const", bufs=1) as cpool, \
             tc.tile_pool(name="msg", bufs=8) as mpool, \
             tc.tile_pool(name="ep", bufs=4) as epool:
            dp = cpool.tile([P, tiles], f32)
            nc.sync.dma_start(out=dp[:], in_=disperm[:, :])
            b2 = cpool.tile([P, HID], f32)
            nc.sync.dma_start(out=b2[:], in_=b2rep[:, :])
            w3 = cpool.tile([P, HID], f32)
            nc.sync.dma_start(out=w3[:], in_=w3drep[:, :])
            ix = cpool.tile([P, R], i32)
            nc.sync.dma_start(out=ix[:], in_=idx2[:, :])
            own = cpool.tile([P, tiles * HID], f32)
            nc.sync.dma_start(out=own[:], in_=own2d[:, :])

            SB = next(k for k in range(min(14, tiles), 0, -1)
                      if tiles % k == 0)
            GW = next(k for k in (7, 2, 1) if tiles % k == 0 and SB % k == 0)
            offs = [0] * (tiles + 1)
            for t in range(tiles):
                offs[t + 1] = offs[t] + int(D[t])
            for t0 in range(0, tiles, SB):
                for j0 in range(0, SB, GW):
                    base = t0 + j0
                    ds = [int(D[base + j]) for j in range(GW)]
                    assert ds == sorted(ds, reverse=True), ds
                    for r in range(1, ds[0]):
                        w = sum(1 for d in ds if d > r)
                        msgw = mpool.tile([P, GW * HID], f32, tag="msgw")
                        for j in range(w):
                            nc.gpsimd.indirect_dma_start(
                                out=msgw[:, j * HID:(j + 1) * HID],
                                out_offset=None, in_=table2[:],
                                in_offset=bass.IndirectOffsetOnAxis(
                                    ap=ix[:, offs[base + j] + r:
                                           offs[base + j] + r + 1], axis=0))
                        gsl = own[:, base * HID:(base + w) * HID]
                        nc.vector.tensor_tensor(out=gsl, in0=gsl,
                                                in1=msgw[:, :w * HID],
                                                op=mybir.AluOpType.add)
                # batched epilogue over SB tiles
                seg3 = own[:, t0 * HID:(t0 + SB) * HID].rearrange(
                    "p (s d) -> p s d", d=HID)
                o2 = epool.tile([P, SB * HID], f32, tag="o2")
                o23 = o2[:].rearrange("p (s d) -> p s d", d=HID)
                nc.vector.tensor_tensor(
                    out=o23, in0=seg3,
                    in1=dp[:, t0:t0 + SB].to_broadcast([P, SB, HID]),
                    op=mybir.AluOpType.mult)
                nc.vector.tensor_tensor(
                    out=o23, in0=o23,
                    in1=b2[:].rearrange("p (o d) -> p o d", o=1)
                          .to_broadcast([P, SB, HID]),
                    op=mybir.AluOpType.add)
                nc.scalar.activation(out=o2[:], in_=o2[:],
                                     func=mybir.ActivationFunctionType.Relu)
                # head: z = sum_d o2*w3diff (+ b3diff); lp0=-sp(z); lp1=z-sp(z)
                tmp = epool.tile([P, SB * HID], f32, tag="tmp")
                nc.vector.tensor_tensor(
                    out=tmp[:].rearrange("p (s d) -> p s d", d=HID), in0=o23,
                    in1=w3[:].rearrange("p (o d) -> p o d", o=1)
                          .to_broadcast([P, SB, HID]),
                    op=mybir.AluOpType.mult)
                z = epool.tile([P, SB], f32, tag="z")
                nc.vector.tensor_reduce(
                    out=z[:].rearrange("p (s o) -> p s o", o=1),
                    in_=tmp[:].rearrange("p (s d) -> p s d", d=HID),
                    axis=mybir.AxisListType.X, op=mybir.AluOpType.add)
                if b3diff != 0.0:
                    nc.vector.tensor_scalar_add(out=z[:], in0=z[:],
                                                scalar1=float(b3diff))
                # stable softplus: m=max(z,0); sp = m + ln(1+exp(z-2m))
                m = epool.tile([P, SB], f32, tag="m")
                nc.vector.tensor_scalar_max(out=m[:], in0=z[:], scalar1=0.0)
                e = epool.tile([P, SB], f32, tag="e")
                nc.vector.tensor_scalar(out=e[:], in0=m[:], scalar1=-2.0,
                                        scalar2=None,
                                        op0=mybir.AluOpType.mult)
                nc.vector.tensor_tensor(out=e[:], in0=e[:], in1=z[:],
                                        op=mybir.AluOpType.add)
                nc.scalar.activation(out=e[:], in_=e[:],
                                     func=mybir.ActivationFunctionType.Exp)
                nc.vector.tensor_scalar_add(out=e[:], in0=e[:], scalar1=1.0)
                nc.scalar.activation(out=e[:], in_=e[:],
                                     func=mybir.ActivationFunctionType.Ln)
                sp = epool.tile([P, SB], f32, tag="sp")
                nc.vector.tensor_tensor(out=sp[:], in0=e[:], in1=m[:],
                                        op=mybir.AluOpType.add)
                lp0 = epool.tile([P, SB], f32, tag="lp0")
                nc.vector.tensor_scalar_mul(out=lp0[:], in0=sp[:],
                                            scalar1=-1.0)
                lp1 = epool.tile([P, SB], f32, tag="lp1")
                nc.vector.tensor_tensor(out=lp1[:], in0=z[:], in1=sp[:],
                                        op=mybir.AluOpType.subtract)
                nc.sync.dma_start(out=outd[:, t0:t0 + SB], in_=lp0[:])
                nc.sync.dma_start(out=outd[:, tiles + t0:tiles + t0 + SB],
                                  in_=lp1[:])
    return nc


# ----------------------------------------------------------------------------
# main entry
# ----------------------------------------------------------------------------
def kernel(x, edge_index, W1, b1, W2, b2, W3, b3):
    _apply_patches()
    x = np.asarray(x, dtype=np.float32)
    n, n_feat = x.shape
    t_start = time.time()
    g = _prep_graph(edge_index, n)
    tiles, slots, D, R = g["tiles"], g["slots"], g["D"], g["R"]
    tiles_all = NCORES * tiles
    nall = tiles_all * P
    _log(f"prep {time.time()-t_start:.1f}s tiles={tiles} R={R} "
         f"pad={(R*P*NCORES)/(edge_index.shape[1]+n)-1:.2%}")

    order, newid = g["order"], g["newid"]
    deg = g["deg"]
    dis = (1.0 / np.sqrt(deg)).astype(np.float32)

    # xT_perm [n_feat, nall]: column (new id) = x[orig]; pads zero
    xTp = np.zeros((n_feat, nall), dtype=np.float32)
    xTp[:, newid] = x.T
    # dis arrays
    disall = np.ones((P, tiles_all), dtype=np.float32)
    disall[newid % P, newid // P] = dis
    disperm = [np.ones((P, tiles), dtype=np.float32) for _ in range(NCORES)]
    for c in range(NCORES):
        nid = g["nid_grid"][c]  # orig ids per slot, -1 pad
        m = nid >= 0
        s = np.arange(slots)[m]
        disperm[c][s % P, s // P] = dis[nid[m]]
    b1rep = np.broadcast_to(np.asarray(b1, np.float32), (P, HID)).copy()
    b2rep = np.broadcast_to(np.asarray(b2, np.float32), (P, HID)).copy()
    w3 = np.asarray(W3, np.float32)
    w3drep = np.broadcast_to((w3[:, 1] - w3[:, 0]), (P, HID)).copy()
    b3 = np.asarray(b3, np.float32)
    b3diff = float(b3[1] - b3[0])

    # gather indices
    idx_new = g["idx_new"]  # [NCORES, P, R] new-ids, -1 pad
    DUMMY1_NEW = slots - 1  # core 0's last pad slot -> zero row (pads exist
    # because slots > n/NCORES; guaranteed since 12544 > 12500)
    assert slots * NCORES > n, "no pad slot available for dummy row"
    i1 = idx_new.copy()
    i1[i1 < 0] = DUMMY1_NEW
    # p-major table1 mapping: row(nid) = (nid%128)*tiles_all + nid//128
    idx1 = ((i1 % P) * tiles_all + i1 // P).astype(np.int32)
    vt2 = NCORES * slots + 1
    i2 = idx_new.copy()
    i2[i2 < 0] = vt2 - 1  # host-appended zero row
    idx2 = i2.astype(np.int32)

    _log(f"host arrays {time.time()-t_start:.1f}s")

    # ---- dispatch A ----
    ncA = _build_A(tiles_all, tiles, D, R, n_feat)
    _log(f"built A {time.time()-t_start:.1f}s")
    rA = _Runner(ncA, replicated=("xT", "W1d", "W2d", "disall", "b1rep",
                                  "identd"))
    inA = [{"xT": xTp,
            "W1d": np.asarray(W1, np.float32),
            "W2d": np.asarray(W2, np.float32),
            "disall": disall,
            "disperm": disperm[c],
            "b1rep": b1rep,
            "identd": np.eye(P, dtype=np.float32),
            "idx1": idx1[c]} for c in range(NCORES)]
    rA.stage(inA)
    _log(f"staged A {time.time()-t_start:.1f}s")
    resA = rA.run()
    _log(f"ran A {time.time()-t_start:.1f}s")

    # host all-gather: assemble table2 [vt2, HID]
    table2 = np.zeros((vt2, HID), dtype=np.float32)
    for c in range(NCORES):
        sh = resA[c]["shard2"].reshape(P, tiles, HID).transpose(1, 0, 2)
        table2[c * slots:(c + 1) * slots] = sh.reshape(slots, HID)
    # zero the pad rows (robustness when b1 != 0)
    pad_mask = np.ones(NCORES * slots, dtype=bool)
    for c in range(NCORES):
        nid = g["nid_grid"][c]
        s = np.arange(slots)[nid >= 0]
        pad_mask[c * slots + s] = False
    table2[:-1][pad_mask] = 0.0

    # ---- dispatch B ----
    ncB = _build_B(tiles, D, R, vt2, b3diff)
    _log(f"built B {time.time()-t_start:.1f}s")
    rB = _Runner(ncB, replicated=("table2", "b2rep", "w3drep"))
    inB = [{"table2": table2,
            "idx2": idx2[c],
            "disperm": disperm[c],
            "b2rep": b2rep,
            "w3drep": w3drep,
            "own2d": resA[c]["shard2"]} for c in range(NCORES)]
    rB.stage(inB)
    _log(f"staged B {time.time()-t_start:.1f}s")
    resB = rB.run()
    _log(f"ran B {time.time()-t_start:.1f}s")

    # ---- unshard: [P, tiles*2] p-major -> [n, 2] in original order ----
    full = np.empty((NCORES * slots, 2), dtype=np.float32)
    for c in range(NCORES):
        o = resB[c]["out"]  # [P, 2*tiles]: lp0 block | lp1 block, p-major
        lp0 = o[:, :tiles].T.reshape(slots)
        lp1 = o[:, tiles:].T.reshape(slots)
        full[c * slots:(c + 1) * slots, 0] = lp0
        full[c * slots:(c + 1) * slots, 1] = lp1
    out = full[newid]
    # keep runners alive for optional re-timing by test harness
    kernel._last = dict(rA=rA, rB=rB)
    _log(f"done {time.time()-t_start:.1f}s")
    return out.astype(np.float32)
